# revision 4
# baseline (speedup 1.0000x reference)
"""Trainium2 Bass kernel for nn_Poolinglabel_91104846282958 (v13): host-transposed input, no final transpose, contiguous output DMA.

Same bitmask OR-tree algorithm as v1, restructured for engine overlap:
  - input DMA on two queues (SP + GpSimd rings), seg order 0,1 first
  - encode: seg0 on DVE, segs 1-3 pass1 on ACT (DVE does seg1 pass2),
    keeping DVE free for the OR trees
  - horizontal tree per segment-pair so PE transposes start early
  - PSUM->SBUF copies on ACT
  - decode: broadcast-AP bitwise AND in 4 class-quarters on DVE,
    int->f16 via ACT Sign (first 3 quarters) / DVE is_gt (last),
    each quarter DMA'd out as soon as converted (SP/PE rings)
"""
import sys

if "/opt/trn_rl_repo" not in sys.path:
    sys.path.insert(0, "/opt/trn_rl_repo")

import numpy as np

B = 8
R = 512          # rows
C = 512          # cols
S = 4            # row segments of 128
P = 128          # partitions
PADL = 19
W = 552          # PADL + 512 + 21 right pad
OC = 128         # output cols
ORR = 128        # output rows
NCLS = 19

_PROGRAM = None


def _uv_op(nc, pool, src, seg0, nseg, uv_dst, zero):
    """First tree level: uv_dst [P, nseg, 2, 137] with
    [..,0,j]=m[4j]|m[4j+1], [..,1,j]=m[4j+2]|m[4j+3]."""
    import concourse.mybir as mybir

    OR_ = mybir.AluOpType.bitwise_or
    base = src[:, seg0 * W : (seg0 + nseg) * W].rearrange("p (s w) -> p s w", w=W)

    def m4(elem_off, num):
        s0 = base[:, :, elem_off : elem_off + 4 * num]
        return (s0.rearrange("p s (a b) -> p s a b", b=4)[:, :, :, 0:3:2]
                .transpose([0, 1, 3, 2]))

    nc.vector.tensor_tensor(uv_dst, m4(0, 137), m4(1, 137), OR_)


def _chain_ops(nc, pool, src, seg0, nseg, uv, dst, tag):
    """Levels 2-8 over nseg segments: uv [P, nseg, 2, 137] -> dst [P,nseg,128]."""
    import concourse.mybir as mybir

    I32 = mybir.dt.int32
    OR_ = mybir.AluOpType.bitwise_or
    g = pool.tile([P, nseg * 136], I32, tag=f"g{tag}")
    d1 = pool.tile([P, nseg * 135], I32, tag=f"d1{tag}")
    s4 = pool.tile([P, nseg * 133], I32, tag=f"s4{tag}")
    z1 = pool.tile([P, nseg * 128], I32, tag=f"z1{tag}")
    z2 = pool.tile([P, nseg * 128], I32, tag=f"z2{tag}")
    z3 = pool.tile([P, nseg * 128], I32, tag=f"z3{tag}")

    base = src[:, seg0 * W : (seg0 + nseg) * W].rearrange("p (s w) -> p s w", w=W)

    def seg(t, width, lo=0, hi=None):
        hi = width if hi is None else hi
        return t[:].rearrange("p (s w) -> p s w", w=width)[:, :, lo:hi]

    u = uv[:, :, 0, :]
    v = uv[:, :, 1, :]
    tt = nc.vector.tensor_tensor
    tt(seg(g, 136), u[:, :, 0:136], v[:, :, 0:136], OR_)
    tt(seg(d1, 135), seg(g, 136, 0, 135), seg(g, 136, 1, 136), OR_)
    tt(seg(s4, 133), seg(d1, 135, 0, 133), seg(d1, 135, 2, 135), OR_)
    tt(seg(z1, 128), seg(s4, 133, 0, 128), seg(s4, 133, 4, 132), OR_)
    tt(seg(z2, 128), seg(z1, 128), seg(g, 136, 8, 136), OR_)
    tt(seg(z3, 128), seg(z2, 128), u[:, :, 9:137], OR_)
    # final level per segment so each transpose can start asap
    for h in range(nseg):
        tt(dst[:, h : h + 1],
           seg(z3, 128)[:, h : h + 1],
           base[:, h : h + 1, 38 : 38 + 4 * 127 + 1 : 4],
           OR_)


def _build_body(tc, y_d, x_d):
    import concourse.mybir as mybir

    nc = tc.nc
    F32 = mybir.dt.float32
    I32 = mybir.dt.int32
    F16 = mybir.dt.float16
    OR_ = mybir.AluOpType.bitwise_or
    AND_ = mybir.AluOpType.bitwise_and
    COPY = mybir.ActivationFunctionType.Copy
    SIGN = mybir.ActivationFunctionType.Sign
    ESC = 8388608.0          # 2^23
    EBI = 1065353216.0       # 0x3F800000 as int

    with tc.tile_pool(name="main", bufs=1) as pool, \
         tc.tile_pool(name="psum", bufs=2, space="PSUM") as psum:
        xin = pool.tile([P, S * C], F32)
        # input DMAs first: SP ring carries segs 0,2; ACT ring segs 1,3
        nc.sync.dma_start(out=xin[:, 0:C], in_=x_d[0:P, :])
        nc.scalar.dma_start(out=xin[:, C : 2 * C], in_=x_d[P : 2 * P, :])
        nc.sync.dma_start(out=xin[:, 2 * C : 3 * C], in_=x_d[2 * P : 3 * P, :])
        nc.scalar.dma_start(out=xin[:, 3 * C : 4 * C], in_=x_d[3 * P : 4 * P, :])

        rid = pool.tile([P, P], F32)
        cid = pool.tile([P, P], F32)
        ident = pool.tile([P, P], F32)
        ciota = pool.tile([P, NCLS], F32)
        nc.gpsimd.iota(rid[:], [[0, P]], channel_multiplier=1,
                       allow_small_or_imprecise_dtypes=True)
        nc.gpsimd.iota(cid[:], [[1, P]], channel_multiplier=0,
                       allow_small_or_imprecise_dtypes=True)
        nc.gpsimd.iota(ciota[:], [[1, NCLS]], channel_multiplier=0,
                       allow_small_or_imprecise_dtypes=True)
        nc.vector.tensor_tensor(ident[:], rid[:], cid[:], mybir.AluOpType.is_equal)

        zero = pool.tile([P, 1], I32)
        nc.vector.memset(zero[:], 0)

        # class bit masks 1<<c via the same two-pass encode, on ACT
        kmask = pool.tile([P, NCLS], I32)
        mask = pool.tile([P, NCLS], I32)
        nc.scalar.activation(kmask[:], ciota[:], COPY, bias=EBI, scale=ESC)
        nc.scalar.activation(mask[:], kmask[:].bitcast(F32), COPY)

        mbuf = pool.tile([P, S * W], I32)
        mb3 = mbuf[:].rearrange("p (s w) -> p s w", w=W)
        nc.gpsimd.memset(mb3[:, :, 0:PADL], 0)
        nc.gpsimd.memset(mb3[:, :, PADL + C : W], 0)

        vbuf2 = pool.tile([P, W], I32)
        nc.gpsimd.memset(vbuf2[:, 0:PADL], 0)
        nc.gpsimd.memset(vbuf2[:, PADL + R : W], 0)

        def mseg(s):
            return (mbuf[:, s * W : (s + 1) * W]
                    .rearrange("p (a w) -> p a w", a=1)[:, :, PADL : PADL + C])

        kbuf = pool.tile([P, 2 * C], I32)

        # encode segs 0,1 fully on DVE (earliest start), segs 2-3 on ACT
        # (overlapping the DVE tree of pair 0)
        nc.vector.tensor_scalar(kbuf[:, 0:C], xin[:, 0:C], ESC, EBI,
                                mybir.AluOpType.mult, mybir.AluOpType.add)
        nc.vector.tensor_copy(mseg(0), kbuf[:, 0:C].bitcast(F32)
                              .rearrange("p (a w) -> p a w", a=1))
        nc.vector.tensor_scalar(kbuf[:, C : 2 * C], xin[:, C : 2 * C], ESC, EBI,
                                mybir.AluOpType.mult, mybir.AluOpType.add)
        nc.vector.tensor_copy(mseg(1), kbuf[:, C : 2 * C].bitcast(F32)
                              .rearrange("p (a w) -> p a w", a=1))

        uv_all = pool.tile([P, S * 2 * 137], I32)
        uv4 = uv_all[:].rearrange("p (s a b) -> p s a b", a=2, b=137)
        hbuf_i = pool.tile([P, S * OC], I32)
        hb3 = hbuf_i[:].rearrange("p (s w) -> p s w", w=OC)

        # pair 0 tree on DVE
        _uv_op(nc, pool, mbuf, 0, 2, uv4[:, 0:2], zero)

        # encode segs 2,3 on ACT (emitted now so it runs under the DVE chain)
        kb2 = pool.tile([P, 2 * C], I32)
        nc.scalar.activation(kb2[:, 0:C], xin[:, 2 * C : 3 * C], COPY,
                             bias=EBI, scale=ESC)
        nc.scalar.activation(mseg(2), kb2[:, 0:C].bitcast(F32)
                             .rearrange("p (a w) -> p a w", a=1), COPY)
        nc.scalar.activation(kb2[:, C : 2 * C], xin[:, 3 * C : 4 * C], COPY,
                             bias=EBI, scale=ESC)
        nc.scalar.activation(mseg(3), kb2[:, C : 2 * C].bitcast(F32)
                             .rearrange("p (a w) -> p a w", a=1), COPY)

        _chain_ops(nc, pool, mbuf, 0, 2, uv4[:, 0:2], hb3[:, 0:2], "p0")

        # transposes of segs 0,1 (PE) + PSUM->SBUF copies (ACT)
        for s in range(2):
            pt = psum.tile([P, P], F32, tag="pt")
            nc.tensor.transpose(pt[:], hbuf_i[:, s * OC : (s + 1) * OC].bitcast(F32),
                                ident[:])
            nc.scalar.copy(vbuf2[:, PADL + s * P : PADL + (s + 1) * P].bitcast(F32),
                           pt[:])

        # pair 1 tree on DVE
        _uv_op(nc, pool, mbuf, 2, 2, uv4[:, 2:4], zero)
        _chain_ops(nc, pool, mbuf, 2, 2, uv4[:, 2:4], hb3[:, 2:4], "p1")

        for s in range(2, 4):
            pt = psum.tile([P, P], F32, tag="pt")
            nc.tensor.transpose(pt[:], hbuf_i[:, s * OC : (s + 1) * OC].bitcast(F32),
                                ident[:])
            nc.scalar.copy(vbuf2[:, PADL + s * P : PADL + (s + 1) * P].bitcast(F32),
                           pt[:])

        # stage-2 tree (DVE). The uv level is split: the left 68 outputs
        # read only cols <272 (transposes 0,1), so they run while the
        # T2/T3 copies are still in flight.
        obuf_i = pool.tile([P, ORR], I32)
        uvv = pool.tile([P, 2 * 137], I32)
        uvv4 = uvv[:].rearrange("p (s a b) -> p s a b", a=2, b=137)
        base_v = vbuf2[:].rearrange("p (s w) -> p s w", w=W)

        def m4v(j0, nj, off):
            s0 = base_v[:, :, 4 * j0 + off : 4 * j0 + off + 4 * nj]
            return (s0.rearrange("p s (a b) -> p s a b", b=4)[:, :, :, 0:3:2]
                    .transpose([0, 1, 3, 2]))

        nc.vector.tensor_tensor(uvv4[:, 0:1, :, 0:68],
                                m4v(0, 68, 0), m4v(0, 68, 1), OR_)
        nc.vector.tensor_tensor(uvv4[:, 0:1, :, 68:137],
                                m4v(68, 69, 0), m4v(68, 69, 1), OR_)
        _chain_ops(nc, pool, vbuf2, 0, 1, uvv4[:, 0:1],
                   obuf_i[:].rearrange("p (s w) -> p s w", w=ORR), "v")

        # decode straight off obuf (already [out-row, out-col] oriented since
        # the input is host-transposed and pooling is symmetric): 19x
        # shift+and on DVE, int->f16 via ACT Sign (q1,q2) / DVE is_gt
        # (q3,q4); each quarter is one contiguous per-partition DMA run
        dec_i = pool.tile([P, NCLS * OC], I32)
        dec = pool.tile([P, NCLS * OC], F16)
        y_flat = y_d.rearrange("p c w -> p (c w)")
        groups = ((0, 5), (5, 10), (10, 15), (15, NCLS))
        dmae = (nc.sync, nc.scalar, nc.sync, nc.scalar)
        for k, (c0, c1) in enumerate(groups):
            for c in range(c0, c1):
                nc.vector.tensor_scalar(dec_i[:, c * OC : (c + 1) * OC], obuf_i[:],
                                        c, 1, mybir.AluOpType.logical_shift_right,
                                        AND_)
            if k < 2:
                nc.scalar.activation(dec[:, c0 * OC : c1 * OC],
                                     dec_i[:, c0 * OC : c1 * OC], SIGN)
            else:
                nc.vector.tensor_scalar(dec[:, c0 * OC : c1 * OC],
                                        dec_i[:, c0 * OC : c1 * OC], 0, None,
                                        mybir.AluOpType.is_gt)
            dmae[k].dma_start(out=y_flat[:, c0 * OC : c1 * OC],
                              in_=dec[:, c0 * OC : c1 * OC])


def _split_waits(nc, maxw=1):
    """The axon/walrus codegen path encodes at most one sync-wait per
    instruction; hoist excess waits onto preceding same-engine NoOps."""
    import concourse.mybir as mybir

    cnt = 0
    for fn in nc.m.functions:
        for blk in fn.blocks:
            newlist = []
            for inst in blk.instructions:
                si = inst.sync_info
                if si and si.on_wait and len(si.on_wait) > maxw:
                    waits = list(si.on_wait)
                    head, tail = waits[:-maxw], waits[-maxw:]
                    k = 0
                    while head:
                        chunk, head = head[:maxw], head[maxw:]
                        n = mybir.InstNoOp(name=f"{inst.name}-w{k}", ins=[], outs=[])
                        n.engine = inst.engine
                        n.sync_info = mybir.SyncInfo(on_wait=chunk, on_update=[])
                        newlist.append(n)
                        cnt += 1
                        k += 1
                    inst.sync_info = mybir.SyncInfo(on_wait=tail,
                                                    on_update=list(si.on_update or []))
                newlist.append(inst)
            blk.instructions[:] = newlist
    return cnt


def _build_program():
    global _PROGRAM
    if _PROGRAM is None:
        import concourse.bass as bass
        import concourse.mybir as mybir
        from concourse.tile import TileContext

        nc = bass.Bass("TRN2", debug=False)
        x_h = nc.declare_dram_parameter("x", [R, C], mybir.dt.float32, isOutput=False)
        y_h = nc.declare_dram_parameter("y", [ORR, NCLS, OC], mybir.dt.float16,
                                        isOutput=True)
        with TileContext(nc) as tc:
            _build_body(tc, y_h.ap(), x_h.ap())
        _split_waits(nc)
        _PROGRAM = nc
    return _PROGRAM


def kernel(x: np.ndarray) -> np.ndarray:
    """x: [8,512,512] float32 class ids -> [8,19,128,128] float16."""
    import time
    from concourse.bass_utils import run_bass_kernel_spmd

    nc = _build_program()
    x = np.asarray(x, dtype=np.float32)
    assert x.shape == (B, R, C), x.shape
    # device pipeline pools the free dim first; feed the transposed image so
    # stage 1 is the vertical pool and stage 2 lands output-oriented
    in_maps = [{"x": np.ascontiguousarray(x[i].T)} for i in range(B)]
    last_err = None
    for attempt in range(3):
        try:
            res = run_bass_kernel_spmd(nc, in_maps, list(range(B)))
            break
        except Exception as e:  # transient NRT device-state hiccups
            last_err = e
            time.sleep(2.0)
    else:
        raise last_err
    # y is stored [row, class, col] on device; restore [class, row, col]
    return np.stack([np.transpose(np.asarray(res.results[i]["y"],
                                             dtype=np.float16), (1, 0, 2))
                     for i in range(B)])
